# revision 32
# baseline (speedup 1.0000x reference)
"""Causal multi-head attention (B=4, S=2048, H=2048, NH=16) on 8 TRN2 NeuronCores.

Strategy (tensor-parallel over heads + all-to-all reshard):
  - Each core owns 2 heads. Host slices W_attn/b_attn per core, casts to
    bf16 and pre-transposes x (fp32 accumulation happens in PSUM).
  - Phase A (per batch): QKV projection from x^T tiles produces Q^T, K^T
    ([head_dim, tokens]) and V ([tokens, head_dim] with interleaved ones
    columns carrying a free softmax denominator).
  - Phase B (per batch): loops q-chunk outer / head inner. kv-blocks are
    paired into [128,1024] PSUM tiles so one ScalarE exp covers two
    blocks (ScalarE is the B-phase co-bottleneck). P^T tiles are PV
    stationary operands, rhs = [V | ones]; normalize on VectorE.
  - TWO AllToAlls per batch (one per 1024-token half), triggered ~30%
    and 100% through phase B: halves each collective's SDMA window (the
    collectives contend with x-tile loads for DMA) and lets the second
    half overlap the next phase with enough x-prefetch to cover it.
  - Phase C: output projection for this core's 2x128-token slices.
  - Queues: sync = x-loads + at-transposes ONLY (never blocked by
    attention stores); scalar = exp + odd an-stores + wproj + C stores;
    gpsimd = even an-stores + collective triggers (SWDGE).

Self-contained: hardcodes all shapes; no file reads.
"""

import numpy as np
import ml_dtypes

import concourse.bacc as bacc
import concourse.tile as tile
import concourse.mybir as mybir
from concourse import bass_utils

BF16 = mybir.dt.bfloat16
F32 = mybir.dt.float32
AF = mybir.ActivationFunctionType

N_CORES = 8
B = 4
S = 2048
H = 2048
NH = 16
HD = 128
HPC = NH // N_CORES          # heads per core = 2
TOK = B * S                  # 8192
KCH = H // 128               # 16 hidden chunks
SC = 512                     # token chunk for projections / q-chunks
TPB_CH = S // SC             # 4 token chunks per batch
QB = S // 128                # 16 q/kv blocks per batch
SCALE = 1.0 / float(np.sqrt(HD))
VSTRIDE = 2 * (HD + 1)       # V storage: per tokblock [Vh0|1|Vh1|1]
TPB = S // N_CORES           # 256 tokens per core per batch after A2A
HT = S // 2                  # tokens per half-batch A2A = 1024

_CACHE: dict = {}
LAST_RESULT = None


def _build(zero_bias=False):
    nc = bacc.Bacc("TRN2", target_bir_lowering=False, debug=False,
                   num_devices=N_CORES)
    xT = nc.dram_tensor("xT", [H, TOK], BF16, kind="ExternalInput")
    wqkv = nc.dram_tensor("wqkv", [H, 6 * HD], BF16, kind="ExternalInput")
    wproj = nc.dram_tensor("wproj", [H, H], BF16, kind="ExternalInput")
    bqkv = nc.dram_tensor("bqkv", [1, 6 * HD], BF16, kind="ExternalInput")
    bqk_t = nc.dram_tensor("bqk_t", [128, 4], F32, kind="ExternalInput")
    bproj = nc.dram_tensor("bproj", [1, H], BF16, kind="ExternalInput")
    mask = nc.dram_tensor("mask", [128, 128], BF16, kind="ExternalInput")
    out = nc.dram_tensor("out", [B * TPB, H], F32, kind="ExternalOutput")

    with tile.TileContext(nc) as tc:
        with (
            tc.tile_pool(name="const", bufs=1) as constp,
            tc.tile_pool(name="qkp", bufs=8) as qkp,
            tc.tile_pool(name="vsp", bufs=2) as vsp,
            tc.tile_pool(name="wpstore", bufs=1) as wpstore,
            tc.tile_pool(name="dram", bufs=1, space="DRAM") as dram,
            tc.tile_pool(name="xTp", bufs=30) as xTp,
            tc.tile_pool(name="wqp", bufs=1) as wqp,
            tc.tile_pool(name="psA", bufs=2, space="PSUM") as psA,
            tc.tile_pool(name="psS", bufs=2, space="PSUM") as psS,
            tc.tile_pool(name="ptP", bufs=10) as ptP,
            tc.tile_pool(name="anP", bufs=6) as anP,
            tc.tile_pool(name="recP", bufs=4) as recP,
            tc.tile_pool(name="atP", bufs=4) as atP,
            tc.tile_pool(name="psC", bufs=2, space="PSUM") as psC,
            tc.tile_pool(name="outP", bufs=2) as outP,
        ):
            mask_sb = constp.tile([128, 128], BF16, name="mask_sb")
            nc.sync.dma_start(mask_sb[:], mask[:])
            ones_sb = constp.tile([1, 128], BF16, name="ones_sb")
            nc.vector.memset(ones_sb[:], 1.0)
            bqkv_sb = constp.tile([1, 6 * HD], BF16, name="bqkv_sb")
            nc.sync.dma_start(bqkv_sb[:], bqkv[:])
            bqkt_sb = constp.tile([128, 4], F32, name="bqkt_sb")
            nc.sync.dma_start(bqkt_sb[:], bqk_t[:])
            bproj_sb = constp.tile([1, H], BF16, name="bproj_sb")
            nc.sync.dma_start(bproj_sb[:], bproj[:])

            # W_qkv resident; tiles interleave with the first x chunk.
            wt = [wqp.tile([128, 6 * HD], BF16, name=f"wt{kc}")
                  for kc in range(KCH)]
            # W_proj resident, loaded on scalar during A0 compute.
            wpt = [wpstore.tile([128, H], BF16, name=f"wpt{kc}")
                   for kc in range(KCH)]

            def load_wproj():
                for kc in range(KCH):
                    nc.scalar.dma_start(wpt[kc][:],
                                        wproj[kc * 128:(kc + 1) * 128, :])

            qk_store = [None] * B
            vstore = [None] * B

            a2a_in = [[dram.tile([HT, HPC * HD], BF16, name=f"cc_in{b}_{hf}")
                       for hf in range(2)] for b in range(B)]
            a2a_out = [[dram.tile([HT, HPC * HD], BF16, name=f"cc_out{b}_{hf}")
                        for hf in range(2)] for b in range(B)]
            # Tiny warm-up collective: pays the cross-core barrier + ncfw
            # init cost during A0 instead of inflating the first real A2A.
            warm_in = dram.tile([N_CORES, 16], BF16, name="warm_in")
            warm_out = dram.tile([N_CORES, 16], BF16, name="warm_out")
            nc.gpsimd.collective_compute(
                "AllToAll", mybir.AluOpType.bypass,
                replica_groups=[list(range(N_CORES))],
                ins=[warm_in.opt()], outs=[warm_out.opt()],
            )

            def phase_a(b):
                """QKV projection for batch b (generator of PE quanta)."""
                qk_store[b] = [qkp.tile([128, S], BF16, name="qkt")
                               for _ in range(4)]
                vstore[b] = vsp.tile([128, QB * VSTRIDE], BF16, name="vst")
                nc.vector.memset(vstore[b][:], 1.0)
                for tloc in range(TPB_CH):
                    t = b * TPB_CH + tloc
                    xt = []
                    for kc in range(KCH):
                        xtile = xTp.tile([128, SC], BF16, name="xt")
                        if b == 0 and tloc == 0:
                            # first chunk: split x across three queues and
                            # interleave W_qkv so both streams land fast
                            eng = (nc.sync if kc < 6 else
                                   nc.scalar if kc < 11 else nc.gpsimd)
                            eng.dma_start(
                                xtile[:],
                                xT[kc * 128:(kc + 1) * 128,
                                   t * SC:(t + 1) * SC])
                            eng.dma_start(
                                wt[kc][:], wqkv[kc * 128:(kc + 1) * 128, :])
                        else:
                            nc.sync.dma_start(
                                xtile[:],
                                xT[kc * 128:(kc + 1) * 128,
                                   t * SC:(t + 1) * SC])
                        xt.append(xtile)
                    for ob in range(4):      # q_h0, q_h1, k_h0, k_h1
                        ps = psA.tile([128, SC], F32, name="psa")
                        for kc in range(KCH):
                            nc.tensor.matmul(
                                ps[:],
                                wt[kc][:, ob * 128:(ob + 1) * 128],
                                xt[kc][:],
                                start=(kc == 0), stop=(kc == KCH - 1))
                        nc.vector.tensor_scalar_add(
                            qk_store[b][ob][:, tloc * SC:(tloc + 1) * SC],
                            ps[:], bqkt_sb[:, ob:ob + 1])
                        yield
                    for tb in range(4):      # V blocks, natural layout
                        psw = psA.tile([128, SC], F32, name="psa")
                        ps = psw[:, 0:2 * HD]
                        for kc in range(KCH):
                            nc.tensor.matmul(
                                ps,
                                xt[kc][:, tb * 128:(tb + 1) * 128],
                                wt[kc][:, 4 * HD:6 * HD],
                                start=(kc == 0),
                                stop=(zero_bias and kc == KCH - 1))
                        if not zero_bias:
                            nc.tensor.matmul(ps, ones_sb[:],
                                             bqkv_sb[:, 4 * HD:6 * HD],
                                             start=False, stop=True)
                        base = (tloc * 4 + tb) * VSTRIDE
                        nc.vector.tensor_copy(
                            vstore[b][:, base:base + HD], ps[:, 0:HD])
                        nc.vector.tensor_copy(
                            vstore[b][:, base + HD + 1:base + 2 * HD + 1],
                            ps[:, HD:2 * HD])
                        if tb % 2 == 1:
                            yield

            def phase_b(b, qc_order=(0, 1, 2, 3)):
                """Attention for batch b (generator of PE quanta);
                q-chunk outer, head inner.

                Triggers the half-batch AllToAll after qc1 and qc3.
                """
                for qc in qc_order:
                    for h in range(HPC):
                        qt = qk_store[b][h]
                        kt = qk_store[b][2 + h]
                        npairs = 2 * (qc + 1)
                        pts = []
                        for pr in range(npairs):
                            kb0 = 2 * pr
                            ps = psS.tile([128, 2 * SC], F32, name="pss")
                            pt = ptP.tile([128, 2 * SC], BF16, name="pt")
                            c0s = [max(0, (kb0 + half) * 128 - qc * SC)
                                   for half in range(2)]
                            for half in range(2):
                                kb = kb0 + half
                                nc.tensor.matmul(
                                    ps[:, half * SC + c0s[half]:
                                       half * SC + SC],
                                    kt[:, kb * 128:(kb + 1) * 128],
                                    qt[:, qc * SC + c0s[half]:(qc + 1) * SC],
                                    start=True, stop=True)
                            if pr < 2 * qc:
                                # full (non-diagonal) pair: one wide exp
                                nc.scalar.activation(
                                    pt[:], ps[:], AF.Exp, scale=SCALE)
                            else:
                                # diagonal pair: per-half exp + mask
                                for half in range(2):
                                    off = half * SC
                                    c0 = c0s[half]
                                    nc.scalar.activation(
                                        pt[:, off + c0:off + SC],
                                        ps[:, off + c0:off + SC],
                                        AF.Exp, scale=SCALE)
                                    nc.vector.tensor_mul(
                                        pt[:, off + c0:off + c0 + 128],
                                        pt[:, off + c0:off + c0 + 128],
                                        mask_sb[:])
                            pts.append(pt)
                            yield
                        for qb in range(4):
                            qg = qc * 4 + qb
                            po = psA.tile([128, SC], F32,
                                          name="psa")[:, 0:HD + 1]
                            for kb in range(qg + 1):
                                vbase = kb * VSTRIDE + h * (HD + 1)
                                src = pts[kb // 2][:, (kb % 2) * SC
                                                   + qb * 128:
                                                   (kb % 2) * SC
                                                   + (qb + 1) * 128]
                                nc.tensor.matmul(
                                    po[:],
                                    src,
                                    vstore[b][:, vbase:vbase + HD + 1],
                                    start=(kb == 0), stop=(kb == qg))
                            rec = recP.tile([128, 1], F32, name="rec")
                            nc.vector.reciprocal(rec[:], po[:, HD:HD + 1])
                            an = anP.tile([128, HD], BF16, name="an")
                            nc.vector.tensor_scalar_mul(
                                an[:], po[:, 0:HD], rec[:])
                            hf = qc // 2
                            row = ((qc % 2) * 4 + qb) * 128
                            eng = nc.gpsimd if qb % 2 == 0 else nc.scalar
                            eng.dma_start(
                                a2a_in[b][hf][row:row + 128,
                                              h * HD:(h + 1) * HD],
                                an[:])
                            yield
                    if qc % 2 == 1:
                        hf = qc // 2
                        nc.gpsimd.collective_compute(
                            "AllToAll",
                            mybir.AluOpType.bypass,
                            replica_groups=[list(range(N_CORES))],
                            ins=[a2a_in[b][hf].opt()],
                            outs=[a2a_out[b][hf].opt()],
                        )

            def phase_c(b):
                """Output projection (generator of PE quanta)."""
                # issue all at-transposes upfront; each waits only on its
                # own A2A half, so early halves transpose while late
                # collectives are still in flight
                at_w = [[None, None], [None, None]]
                for hf in range(2):
                    for fh in range(2):
                        atile = atP.tile([128, HT], BF16, name="at")
                        nc.sync.dma_start(
                            atile[:],
                            a2a_out[b][hf][:, fh * 128:(fh + 1) * 128],
                            transpose=True)
                        at_w[hf][fh] = atile
                for hf in range(2):
                    for oc in range(4):
                        ps = psC.tile([128, SC], F32, name="psc")
                        for hc in range(KCH):
                            nc.tensor.matmul(
                                ps[:],
                                at_w[hf][hc % 2][:, (hc // 2) * 128:
                                                 (hc // 2 + 1) * 128],
                                wpt[hc][:, oc * SC:(oc + 1) * SC],
                                start=(hc == 0),
                                stop=(zero_bias and hc == KCH - 1))
                        if not zero_bias:
                            nc.tensor.matmul(
                                ps[:], ones_sb[:],
                                bproj_sb[:, oc * SC:(oc + 1) * SC],
                                start=False, stop=True)
                        ot = outP.tile([128, SC], F32, name="ot")
                        nc.vector.tensor_copy(ot[:], ps[:])
                        nc.scalar.dma_start(
                            out[b * TPB + hf * 128:b * TPB + (hf + 1) * 128,
                                oc * SC:(oc + 1) * SC],
                            ot[:])
                        yield

            def drain(g):
                for _ in g:
                    pass

            def interleave(ga, gb, na, nb):
                """Alternate na quanta from ga with nb from gb.

                B's short LDWEIGHTS-bound PV matmuls interleave into
                A/C's long streaming matmuls so the PE's reorder window
                hides the weight loads.
                """
                da = db = False
                while not (da and db):
                    for _ in range(na):
                        try:
                            next(ga)
                        except StopIteration:
                            da = True
                            break
                    for _ in range(nb):
                        try:
                            next(gb)
                        except StopIteration:
                            db = True
                            break

            def chain(*gens):
                for g in gens:
                    yield from g

            drain(phase_a(0))
            load_wproj()
            interleave(phase_a(1), phase_b(0), 1, 3)
            interleave(phase_a(2), phase_b(1), 1, 3)
            interleave(phase_a(3), phase_b(2), 1, 3)
            # b3 runs its big q-chunks first so its second-half A2A
            # triggers ~30% earlier; C0+C1 both weave into its stream
            interleave(chain(phase_c(0), phase_c(1)),
                       phase_b(3, qc_order=(2, 3, 0, 1)), 1, 4)
            drain(phase_c(2))
            drain(phase_c(3))

    nc.compile()
    return nc


def _get_nc(zero_bias):
    key = ("nc", bool(zero_bias))
    if key not in _CACHE:
        _CACHE[key] = _build(zero_bias=zero_bias)
    return _CACHE[key]


def kernel(hidden_states, W_attn, b_attn, W_proj, b_proj):
    global LAST_RESULT
    bf = ml_dtypes.bfloat16
    x = np.asarray(hidden_states, dtype=np.float32).reshape(TOK, H)
    xb = x.astype(bf)
    xT = np.ascontiguousarray(xb.view(np.uint16).T).view(bf)
    Wa = np.asarray(W_attn, dtype=np.float32)
    ba = np.asarray(b_attn, dtype=np.float32)
    Wp = np.ascontiguousarray(np.asarray(W_proj, dtype=np.float32)).astype(bf)
    bp = np.asarray(b_proj, dtype=np.float32).reshape(1, H).astype(bf)
    mask = np.triu(np.ones((128, 128), dtype=np.float32)).astype(bf)

    in_maps = []
    for c in range(N_CORES):
        h0 = c * HPC
        cols = []
        for part in range(3):          # q, k, v feature slices
            cols.append(np.arange(part * H + h0 * HD,
                                  part * H + (h0 + HPC) * HD))
        cols = np.concatenate(cols)    # 768 column indices
        wq = np.ascontiguousarray(Wa[:, cols]).astype(bf)
        bq = ba[cols].reshape(1, 6 * HD).astype(bf)
        bqk_t = np.ascontiguousarray(
            ba[cols[:4 * 128]].reshape(4, 128).T).astype(np.float32)
        in_maps.append({
            "xT": xT,
            "wqkv": wq,
            "wproj": Wp,
            "bqkv": bq,
            "bqk_t": bqk_t,
            "bproj": bp,
            "mask": mask,
        })

    zb = (not np.any(np.asarray(b_attn))) and (not np.any(np.asarray(b_proj)))
    nc = _get_nc(zb)
    res = bass_utils.run_bass_kernel_spmd(
        nc, in_maps, core_ids=list(range(N_CORES)))
    LAST_RESULT = res

    # Core c's out rows (b*256 + hf*128 + p) hold batch-b tokens
    # hf*1024 + c*128 + p  (half-batch A2A shard mapping).
    full = np.empty((B, S, H), dtype=np.float32)
    for c in range(N_CORES):
        r = res.results[c]["out"]
        for b in range(B):
            for hf in range(2):
                full[b, hf * HT + c * 128: hf * HT + (c + 1) * 128, :] = \
                    r[b * TPB + hf * 128: b * TPB + (hf + 1) * 128, :]
    return full


# revision 33
# speedup vs baseline: 1.0120x; 1.0120x over previous
"""Causal multi-head attention (B=4, S=2048, H=2048, NH=16) on 8 TRN2 NeuronCores.

Strategy (tensor-parallel over heads + all-to-all reshard):
  - Each core owns 2 heads. Host slices W_attn/b_attn per core, casts to
    bf16 and pre-transposes x (fp32 accumulation happens in PSUM).
  - Phase A (per batch): QKV projection from x^T tiles produces Q^T, K^T
    ([head_dim, tokens]) and V ([tokens, head_dim] with interleaved ones
    columns carrying a free softmax denominator).
  - Phase B (per batch): loops q-chunk outer / head inner. kv-blocks are
    paired into [128,1024] PSUM tiles so one ScalarE exp covers two
    blocks (ScalarE is the B-phase co-bottleneck). P^T tiles are PV
    stationary operands, rhs = [V | ones]; normalize on VectorE.
  - TWO AllToAlls per batch (one per 1024-token half), triggered ~30%
    and 100% through phase B: halves each collective's SDMA window (the
    collectives contend with x-tile loads for DMA) and lets the second
    half overlap the next phase with enough x-prefetch to cover it.
  - Phase C: output projection for this core's 2x128-token slices.
  - Queues: sync = x-loads + at-transposes ONLY (never blocked by
    attention stores); scalar = exp + odd an-stores + wproj + C stores;
    gpsimd = even an-stores + collective triggers (SWDGE).

Self-contained: hardcodes all shapes; no file reads.
"""

import numpy as np
import ml_dtypes

import concourse.bacc as bacc
import concourse.tile as tile
import concourse.mybir as mybir
from concourse import bass_utils

BF16 = mybir.dt.bfloat16
F32 = mybir.dt.float32
AF = mybir.ActivationFunctionType

N_CORES = 8
B = 4
S = 2048
H = 2048
NH = 16
HD = 128
HPC = NH // N_CORES          # heads per core = 2
TOK = B * S                  # 8192
KCH = H // 128               # 16 hidden chunks
SC = 512                     # token chunk for projections / q-chunks
TPB_CH = S // SC             # 4 token chunks per batch
QB = S // 128                # 16 q/kv blocks per batch
SCALE = 1.0 / float(np.sqrt(HD))
VSTRIDE = 2 * (HD + 1)       # V storage: per tokblock [Vh0|1|Vh1|1]
TPB = S // N_CORES           # 256 tokens per core per batch after A2A
HT = S // 2                  # tokens per half-batch A2A = 1024

_CACHE: dict = {}
LAST_RESULT = None


def _build(zero_bias=False):
    nc = bacc.Bacc("TRN2", target_bir_lowering=False, debug=False,
                   num_devices=N_CORES)
    xT = nc.dram_tensor("xT", [H, TOK], BF16, kind="ExternalInput")
    wqkv = nc.dram_tensor("wqkv", [H, 6 * HD], BF16, kind="ExternalInput")
    wproj = nc.dram_tensor("wproj", [H, H], BF16, kind="ExternalInput")
    bqkv = nc.dram_tensor("bqkv", [1, 6 * HD], BF16, kind="ExternalInput")
    bqk_t = nc.dram_tensor("bqk_t", [128, 4], F32, kind="ExternalInput")
    bproj = nc.dram_tensor("bproj", [1, H], BF16, kind="ExternalInput")
    mask = nc.dram_tensor("mask", [128, 128], BF16, kind="ExternalInput")
    out = nc.dram_tensor("out", [B * TPB, H], F32, kind="ExternalOutput")

    with tile.TileContext(nc) as tc:
        with (
            tc.tile_pool(name="const", bufs=1) as constp,
            tc.tile_pool(name="qkp", bufs=8) as qkp,
            tc.tile_pool(name="vsp", bufs=2) as vsp,
            tc.tile_pool(name="wpstore", bufs=1) as wpstore,
            tc.tile_pool(name="dram", bufs=1, space="DRAM") as dram,
            tc.tile_pool(name="xTp", bufs=30) as xTp,
            tc.tile_pool(name="wqp", bufs=1) as wqp,
            tc.tile_pool(name="psA", bufs=2, space="PSUM") as psA,
            tc.tile_pool(name="psS", bufs=2, space="PSUM") as psS,
            tc.tile_pool(name="ptP", bufs=10) as ptP,
            tc.tile_pool(name="anP", bufs=6) as anP,
            tc.tile_pool(name="recP", bufs=4) as recP,
            tc.tile_pool(name="atP", bufs=4) as atP,
            tc.tile_pool(name="psC", bufs=2, space="PSUM") as psC,
            tc.tile_pool(name="outP", bufs=2) as outP,
        ):
            mask_sb = constp.tile([128, 128], BF16, name="mask_sb")
            nc.sync.dma_start(mask_sb[:], mask[:])
            ones_sb = constp.tile([1, 128], BF16, name="ones_sb")
            nc.vector.memset(ones_sb[:], 1.0)
            bqkv_sb = constp.tile([1, 6 * HD], BF16, name="bqkv_sb")
            nc.sync.dma_start(bqkv_sb[:], bqkv[:])
            bqkt_sb = constp.tile([128, 4], F32, name="bqkt_sb")
            nc.sync.dma_start(bqkt_sb[:], bqk_t[:])
            bproj_sb = constp.tile([1, H], BF16, name="bproj_sb")
            nc.sync.dma_start(bproj_sb[:], bproj[:])

            # W_qkv resident; tiles interleave with the first x chunk.
            wt = [wqp.tile([128, 6 * HD], BF16, name=f"wt{kc}")
                  for kc in range(KCH)]
            # W_proj resident, loaded on scalar during A0 compute.
            wpt = [wpstore.tile([128, H], BF16, name=f"wpt{kc}")
                   for kc in range(KCH)]

            def load_wproj():
                for kc in range(KCH):
                    nc.scalar.dma_start(wpt[kc][:],
                                        wproj[kc * 128:(kc + 1) * 128, :])

            qk_store = [None] * B
            vstore = [None] * B

            a2a_in = [[dram.tile([HT, HPC * HD], BF16, name=f"cc_in{b}_{hf}")
                       for hf in range(2)] for b in range(B)]
            a2a_out = [[dram.tile([HT, HPC * HD], BF16, name=f"cc_out{b}_{hf}")
                        for hf in range(2)] for b in range(B)]
            # Tiny warm-up collective: pays the cross-core barrier + ncfw
            # init cost during A0 instead of inflating the first real A2A.
            warm_in = dram.tile([N_CORES, 16], BF16, name="warm_in")
            warm_out = dram.tile([N_CORES, 16], BF16, name="warm_out")
            nc.gpsimd.collective_compute(
                "AllToAll", mybir.AluOpType.bypass,
                replica_groups=[list(range(N_CORES))],
                ins=[warm_in.opt()], outs=[warm_out.opt()],
            )

            def phase_a(b):
                """QKV projection for batch b (generator of PE quanta)."""
                qk_store[b] = [qkp.tile([128, S], BF16, name="qkt")
                               for _ in range(4)]
                vstore[b] = vsp.tile([128, QB * VSTRIDE], BF16, name="vst")
                nc.vector.memset(vstore[b][:], 1.0)
                for tloc in range(TPB_CH):
                    t = b * TPB_CH + tloc
                    xt = []
                    for kc in range(KCH):
                        xtile = xTp.tile([128, SC], BF16, name="xt")
                        if b == 0 and tloc == 0:
                            # first chunk: split x across three queues and
                            # interleave W_qkv so both streams land fast
                            eng = (nc.sync if kc < 6 else
                                   nc.scalar if kc < 11 else nc.gpsimd)
                            eng.dma_start(
                                xtile[:],
                                xT[kc * 128:(kc + 1) * 128,
                                   t * SC:(t + 1) * SC])
                            eng.dma_start(
                                wt[kc][:], wqkv[kc * 128:(kc + 1) * 128, :])
                        else:
                            nc.sync.dma_start(
                                xtile[:],
                                xT[kc * 128:(kc + 1) * 128,
                                   t * SC:(t + 1) * SC])
                        xt.append(xtile)
                    for ob in range(4):      # q_h0, q_h1, k_h0, k_h1
                        ps = psA.tile([128, SC], F32, name="psa")
                        for kc in range(KCH):
                            nc.tensor.matmul(
                                ps[:],
                                wt[kc][:, ob * 128:(ob + 1) * 128],
                                xt[kc][:],
                                start=(kc == 0), stop=(kc == KCH - 1))
                        nc.vector.tensor_scalar_add(
                            qk_store[b][ob][:, tloc * SC:(tloc + 1) * SC],
                            ps[:], bqkt_sb[:, ob:ob + 1])
                        yield
                    for tb in range(4):      # V blocks, natural layout
                        psw = psA.tile([128, SC], F32, name="psa")
                        ps = psw[:, 0:2 * HD]
                        for kc in range(KCH):
                            nc.tensor.matmul(
                                ps,
                                xt[kc][:, tb * 128:(tb + 1) * 128],
                                wt[kc][:, 4 * HD:6 * HD],
                                start=(kc == 0),
                                stop=(zero_bias and kc == KCH - 1))
                        if not zero_bias:
                            nc.tensor.matmul(ps, ones_sb[:],
                                             bqkv_sb[:, 4 * HD:6 * HD],
                                             start=False, stop=True)
                        base = (tloc * 4 + tb) * VSTRIDE
                        nc.vector.tensor_copy(
                            vstore[b][:, base:base + HD], ps[:, 0:HD])
                        nc.vector.tensor_copy(
                            vstore[b][:, base + HD + 1:base + 2 * HD + 1],
                            ps[:, HD:2 * HD])
                        if tb % 2 == 1:
                            yield

            def phase_b(b, qc_order=(0, 1, 2, 3)):
                """Attention for batch b (generator of PE quanta);
                q-chunk outer, head inner.

                Triggers the half-batch AllToAll after qc1 and qc3.
                """
                for qc in qc_order:
                    for h in range(HPC):
                        qt = qk_store[b][h]
                        kt = qk_store[b][2 + h]
                        npairs = 2 * (qc + 1)
                        pts = []
                        for pr in range(npairs):
                            kb0 = 2 * pr
                            ps = psS.tile([128, 2 * SC], F32, name="pss")
                            pt = ptP.tile([128, 2 * SC], BF16, name="pt")
                            c0s = [max(0, (kb0 + half) * 128 - qc * SC)
                                   for half in range(2)]
                            for half in range(2):
                                kb = kb0 + half
                                nc.tensor.matmul(
                                    ps[:, half * SC + c0s[half]:
                                       half * SC + SC],
                                    kt[:, kb * 128:(kb + 1) * 128],
                                    qt[:, qc * SC + c0s[half]:(qc + 1) * SC],
                                    start=True, stop=True)
                            if pr < 2 * qc:
                                # full (non-diagonal) pair: one wide exp
                                nc.scalar.activation(
                                    pt[:], ps[:], AF.Exp, scale=SCALE)
                            else:
                                # diagonal pair: per-half exp + mask
                                for half in range(2):
                                    off = half * SC
                                    c0 = c0s[half]
                                    nc.scalar.activation(
                                        pt[:, off + c0:off + SC],
                                        ps[:, off + c0:off + SC],
                                        AF.Exp, scale=SCALE)
                                    nc.vector.tensor_mul(
                                        pt[:, off + c0:off + c0 + 128],
                                        pt[:, off + c0:off + c0 + 128],
                                        mask_sb[:])
                            pts.append(pt)
                            yield
                        for qb in range(4):
                            qg = qc * 4 + qb
                            po = psA.tile([128, SC], F32,
                                          name="psa")[:, 0:HD + 1]
                            for kb in range(qg + 1):
                                vbase = kb * VSTRIDE + h * (HD + 1)
                                src = pts[kb // 2][:, (kb % 2) * SC
                                                   + qb * 128:
                                                   (kb % 2) * SC
                                                   + (qb + 1) * 128]
                                nc.tensor.matmul(
                                    po[:],
                                    src,
                                    vstore[b][:, vbase:vbase + HD + 1],
                                    start=(kb == 0), stop=(kb == qg))
                            rec = recP.tile([128, 1], F32, name="rec")
                            nc.vector.reciprocal(rec[:], po[:, HD:HD + 1])
                            an = anP.tile([128, HD], BF16, name="an")
                            nc.vector.tensor_scalar_mul(
                                an[:], po[:, 0:HD], rec[:])
                            hf = qc // 2
                            row = ((qc % 2) * 4 + qb) * 128
                            eng = nc.gpsimd if qb % 2 == 0 else nc.scalar
                            eng.dma_start(
                                a2a_in[b][hf][row:row + 128,
                                              h * HD:(h + 1) * HD],
                                an[:])
                            yield
                    if qc % 2 == 1:
                        hf = qc // 2
                        nc.gpsimd.collective_compute(
                            "AllToAll",
                            mybir.AluOpType.bypass,
                            replica_groups=[list(range(N_CORES))],
                            ins=[a2a_in[b][hf].opt()],
                            outs=[a2a_out[b][hf].opt()],
                        )

            def phase_c(b):
                """Output projection (generator of PE quanta)."""
                # issue all at-transposes upfront; each waits only on its
                # own A2A half, so early halves transpose while late
                # collectives are still in flight
                at_w = [[None, None], [None, None]]
                for hf in range(2):
                    for fh in range(2):
                        atile = atP.tile([128, HT], BF16, name="at")
                        nc.sync.dma_start(
                            atile[:],
                            a2a_out[b][hf][:, fh * 128:(fh + 1) * 128],
                            transpose=True)
                        at_w[hf][fh] = atile
                for hf in range(2):
                    for oc in range(4):
                        ps = psC.tile([128, SC], F32, name="psc")
                        for hc in range(KCH):
                            nc.tensor.matmul(
                                ps[:],
                                at_w[hf][hc % 2][:, (hc // 2) * 128:
                                                 (hc // 2 + 1) * 128],
                                wpt[hc][:, oc * SC:(oc + 1) * SC],
                                start=(hc == 0),
                                stop=(zero_bias and hc == KCH - 1))
                        if not zero_bias:
                            nc.tensor.matmul(
                                ps[:], ones_sb[:],
                                bproj_sb[:, oc * SC:(oc + 1) * SC],
                                start=False, stop=True)
                        ot = outP.tile([128, SC], F32, name="ot")
                        nc.vector.tensor_copy(ot[:], ps[:])
                        nc.scalar.dma_start(
                            out[b * TPB + hf * 128:b * TPB + (hf + 1) * 128,
                                oc * SC:(oc + 1) * SC],
                            ot[:])
                        yield

            def drain(g):
                for _ in g:
                    pass

            def interleave(ga, gb, na, nb):
                """Alternate na quanta from ga with nb from gb.

                B's short LDWEIGHTS-bound PV matmuls interleave into
                A/C's long streaming matmuls so the PE's reorder window
                hides the weight loads.
                """
                da = db = False
                while not (da and db):
                    for _ in range(na):
                        try:
                            next(ga)
                        except StopIteration:
                            da = True
                            break
                    for _ in range(nb):
                        try:
                            next(gb)
                        except StopIteration:
                            db = True
                            break

            def chain(*gens):
                for g in gens:
                    yield from g

            drain(phase_a(0))
            load_wproj()
            interleave(phase_a(1), phase_b(0), 1, 3)
            interleave(phase_a(2), phase_b(1), 1, 3)
            interleave(phase_a(3), phase_b(2), 1, 3)
            # C0..C2 weave into B3's stream, B-quanta first so the PE
            # never waits on C0's at-transposes at block start; C3 drains
            # last, consuming hf0 while hf1's collective is in flight
            interleave(phase_b(3),
                       chain(phase_c(0), phase_c(1), phase_c(2)), 4, 1)
            drain(phase_c(3))

    nc.compile()
    return nc


def _get_nc(zero_bias):
    key = ("nc", bool(zero_bias))
    if key not in _CACHE:
        _CACHE[key] = _build(zero_bias=zero_bias)
    return _CACHE[key]


def kernel(hidden_states, W_attn, b_attn, W_proj, b_proj):
    global LAST_RESULT
    bf = ml_dtypes.bfloat16
    x = np.asarray(hidden_states, dtype=np.float32).reshape(TOK, H)
    xb = x.astype(bf)
    xT = np.ascontiguousarray(xb.view(np.uint16).T).view(bf)
    Wa = np.asarray(W_attn, dtype=np.float32)
    ba = np.asarray(b_attn, dtype=np.float32)
    Wp = np.ascontiguousarray(np.asarray(W_proj, dtype=np.float32)).astype(bf)
    bp = np.asarray(b_proj, dtype=np.float32).reshape(1, H).astype(bf)
    mask = np.triu(np.ones((128, 128), dtype=np.float32)).astype(bf)

    in_maps = []
    for c in range(N_CORES):
        h0 = c * HPC
        cols = []
        for part in range(3):          # q, k, v feature slices
            cols.append(np.arange(part * H + h0 * HD,
                                  part * H + (h0 + HPC) * HD))
        cols = np.concatenate(cols)    # 768 column indices
        wq = np.ascontiguousarray(Wa[:, cols]).astype(bf)
        bq = ba[cols].reshape(1, 6 * HD).astype(bf)
        bqk_t = np.ascontiguousarray(
            ba[cols[:4 * 128]].reshape(4, 128).T).astype(np.float32)
        in_maps.append({
            "xT": xT,
            "wqkv": wq,
            "wproj": Wp,
            "bqkv": bq,
            "bqk_t": bqk_t,
            "bproj": bp,
            "mask": mask,
        })

    zb = (not np.any(np.asarray(b_attn))) and (not np.any(np.asarray(b_proj)))
    nc = _get_nc(zb)
    res = bass_utils.run_bass_kernel_spmd(
        nc, in_maps, core_ids=list(range(N_CORES)))
    LAST_RESULT = res

    # Core c's out rows (b*256 + hf*128 + p) hold batch-b tokens
    # hf*1024 + c*128 + p  (half-batch A2A shard mapping).
    full = np.empty((B, S, H), dtype=np.float32)
    for c in range(N_CORES):
        r = res.results[c]["out"]
        for b in range(B):
            for hf in range(2):
                full[b, hf * HT + c * 128: hf * HT + (c + 1) * 128, :] = \
                    r[b * TPB + hf * 128: b * TPB + (hf + 1) * 128, :]
    return full
